# revision 8
# baseline (speedup 1.0000x reference)
"""BERT-CRF Viterbi decode kernel for Trainium2 (Bass/Tile), 8-core data parallel.

Full inputs in, full outputs out. Internally shards batch B=64 across 8 cores
(8 sequences each). Per core, with scan rows r = b*16 + c (c = chunk of 32
timesteps):

  Stage A (u-tiled, fused with scan phase 1):
    for each scan step u (0..31), load sentences for all 128 rows at local
    step u, transpose h-chunks on PE, batched matmul (lhsT = W^T chunk [128,4],
    rhs = 4 steps' transposed sentences [128,512]) -> emissions^T in PSUM,
    fix-transpose back to [rows, 4], write directly into the SBUF scan tile.
    Phase 1 (chunk transfer-matrix recurrence) consumes each step's emissions
    as they land, hidden under stage A's PE/DMA time.
  Phase 2: boundary scores across chunks (sequential over 16, rows 0..7).
  Phase 3: all scores from boundaries + stored prefix matrices (2 big ops).
  Phase 4: backpointer one-hots, first-argmax semantics (6 big ops).
  Phase 5: one-hot matrix backtracking (no gathers).
"""
import sys
for p in ("/opt/trn_rl_repo", "/root/.axon_site/_ro/trn_rl_repo"):
    if p not in sys.path:
        sys.path.append(p)

import numpy as np
import concourse.bass as bass
import concourse.tile as tile
from concourse import mybir
from concourse.bass_utils import run_bass_kernel_spmd

F32 = mybir.dt.float32
I32 = mybir.dt.int32
AX = mybir.AxisListType
OP = mybir.AluOpType

B, T, H, K = 64, 512, 768, 4
NCORES = 8
BC = B // NCORES          # 8 sequences per core
C, L = 16, 32             # chunks per sequence, steps per chunk
ROWS = BC * C             # 128 partition rows
HCH = H // 128            # 6 h-chunks
UG = 4                    # steps per u-group (batched matmul width 4*128=512)

_NC_CACHE = {}


def build_nc():
    nc = bass.Bass()
    sent = nc.declare_dram_parameter("sentences", [BC, T, H], F32, isOutput=False)
    Wd = nc.declare_dram_parameter("W", [K, H], F32, isOutput=False)
    bd = nc.declare_dram_parameter("b", [K], F32, isOutput=False)
    startd = nc.declare_dram_parameter("start_transitions", [K], F32, isOutput=False)
    endd = nc.declare_dram_parameter("end_transitions", [K], F32, isOutput=False)
    transd = nc.declare_dram_parameter("transitions", [K, K], F32, isOutput=False)
    # consts: identity128 (128*128) ++ wfirst4 [4,3,2,1] ++ iw4 [0,1,2,3] ++ ident4 (16)
    constsd = nc.declare_dram_parameter("consts", [128 * 128 + 24], F32, isOutput=False)
    tinitd = nc.declare_dram_parameter("tinit", [128, 16], F32, isOutput=False)
    tagsd = nc.declare_dram_parameter("tags", [BC, T], I32, isOutput=True)

    with tile.TileContext(nc) as tc:
        with tc.tile_pool(name="singles", bufs=1) as singles, \
             tc.tile_pool(name="sent_pool", bufs=6) as sent_pool, \
             tc.tile_pool(name="st_pool", bufs=2) as st_pool, \
             tc.tile_pool(name="tmp_pool", bufs=2) as tmp_pool, \
             tc.tile_pool(name="ps_tr", bufs=4, space="PSUM") as ps_tr, \
             tc.tile_pool(name="ps_eT", bufs=2, space="PSUM") as ps_eT, \
             tc.tile_pool(name="ps_fix", bufs=2, space="PSUM") as ps_fix:

            # ---------- constants ----------
            ident = singles.tile([128, 128], F32)
            nc.sync.dma_start(ident, constsd[:][0:128 * 128].rearrange("(p f) -> p f", p=128))
            wfirst = singles.tile([128, 4], F32)
            nc.sync.dma_start(wfirst, constsd[:][128 * 128:128 * 128 + 4][None, :].to_broadcast((128, 4)))
            iw4 = singles.tile([128, 4], F32)
            nc.sync.dma_start(iw4, constsd[:][128 * 128 + 4:128 * 128 + 8][None, :].to_broadcast((128, 4)))
            id4 = singles.tile([128, 16], F32)
            nc.sync.dma_start(id4, constsd[:][128 * 128 + 8:128 * 128 + 24][None, :].to_broadcast((128, 16)))
            end_sb = singles.tile([128, 4], F32)
            nc.sync.dma_start(end_sb, endd[:][None, :].to_broadcast((128, 4)))
            ttr = singles.tile([128, 16], F32)
            nc.sync.dma_start(ttr, transd[:].rearrange("i j -> (i j)")[None, :].to_broadcast((128, 16)))
            tinit = singles.tile([128, 16], F32)
            nc.sync.dma_start(tinit, tinitd[:])
            b_sb = singles.tile([1, 4], F32)
            nc.sync.dma_start(b_sb, bd[:][None, :])
            ones_sb = singles.tile([1, UG * 128], F32)
            nc.vector.memset(ones_sb, 1.0)

            # ---------- W^T in SBUF: wt[p = h within chunk, ch, k] ----------
            w_raw = singles.tile([K, H], F32)
            nc.sync.dma_start(w_raw, Wd[:])
            wt_sb = singles.tile([128, HCH, K], F32)
            for ch in range(HCH):
                wt_ps = ps_fix.tile([128, K], F32, tag="fix")
                nc.tensor.transpose(wt_ps, w_raw[:, ch * 128:(ch + 1) * 128], ident[0:K, 0:K])
                nc.scalar.copy(wt_sb[:, ch, :], wt_ps)

            # scan emissions tile, written directly by stage A
            emsc = singles.tile([128, L * K], F32)
            emv = emsc.rearrange("p (u j) -> p u j", u=L)

            # phase-1 state: prefix transfer matrices Apre[row, u, i, j]
            Apre = singles.tile([128, L, 4, 4], F32)

            # views
            ttrT_v = ttr.rearrange("p (k j) -> p k j", k=4).transpose([0, 2, 1])  # [p,j,k] = trans[k,j]
            ttr_ji = ttr.rearrange("p (i j) -> p i j", i=4).transpose([0, 2, 1])  # [p,j,i] = trans[i,j]

            # ---------- Stage A (u-tiled) fused with phase 1 ----------
            sA = nc.named_scope("stageA")
            sA.__enter__()
            for g in range(L // UG):
                sents = []
                for uu in range(UG):
                    u = g * UG + uu
                    s_sb = sent_pool.tile([128, H], F32)
                    # row (b*16+c) <- sentences[b, c*32 + u, :]
                    src = bass.AP(
                        tensor=sent[:].tensor, offset=u * H,
                        ap=[[T * H, BC], [L * H, C], [1, H]])
                    nc.sync.dma_start(s_sb, src)
                    sents.append(s_sb)
                # transposes: sT[p=h, ch, uu, rows]
                sT_sb = st_pool.tile([128, HCH, UG, 128], F32)
                for ch in range(HCH):
                    for uu in range(0, UG, 2):
                        trp = ps_tr.tile([128, 256], F32, tag="trps")
                        nc.tensor.transpose(
                            trp[:, 0:128], sents[uu][:, ch * 128:(ch + 1) * 128], ident)
                        nc.tensor.transpose(
                            trp[:, 128:256], sents[uu + 1][:, ch * 128:(ch + 1) * 128], ident)
                        nc.scalar.copy(
                            sT_sb[:, ch, uu:uu + 2, :].rearrange("p a b -> p (a b)"), trp)
                # batched matmuls: out eT[k, uu*128+row] accum over ch, + bias
                eT_ps = ps_eT.tile([4, UG * 128], F32, tag="eT")
                for ch in range(HCH):
                    nc.tensor.matmul(
                        eT_ps, wt_sb[:, ch, :],
                        sT_sb[:, ch, :, :].rearrange("p a b -> p (a b)"),
                        start=(ch == 0), stop=False)
                nc.tensor.matmul(eT_ps, b_sb, ones_sb, start=False, stop=True)
                eT_sb = st_pool.tile([4, UG * 128], F32, tag="eTsb")
                nc.scalar.copy(eT_sb, eT_ps)
                # fix-transpose each uu back to [rows, 4] and land in emsc
                for uu in range(UG):
                    u = g * UG + uu
                    fx = ps_fix.tile([128, K], F32, tag="fix")
                    nc.tensor.transpose(
                        fx, eT_sb[:, uu * 128:(uu + 1) * 128], ident[0:K, 0:K])
                    nc.scalar.copy(emsc[:, u * 4:(u + 1) * 4], fx)
                # ---- phase 1 steps for this group ----
                for uu in range(UG):
                    u = g * UG + uu
                    if u == 0:
                        nc.vector.tensor_tensor(
                            Apre[:, 0, :, :],
                            tinit.rearrange("p (i j) -> p i j", i=4),
                            emv[:, 0, :].unsqueeze(1).to_broadcast((128, 4, 4)),
                            OP.add)
                    else:
                        p1tmp = tmp_pool.tile([128, 4, 4, 4], F32, tag="p1tmp")
                        # tmp[i,j,k] = A[i,k] + trans[k,j]
                        nc.vector.tensor_tensor(
                            p1tmp,
                            Apre[:, u - 1, :, :].unsqueeze(2).to_broadcast((128, 4, 4, 4)),
                            ttrT_v.unsqueeze(1).to_broadcast((128, 4, 4, 4)),
                            OP.add)
                        p1red = tmp_pool.tile([128, 4, 4], F32, tag="p1red")
                        nc.vector.reduce_max(p1red, p1tmp, axis=AX.X)
                        nc.vector.tensor_tensor(
                            Apre[:, u, :, :], p1red,
                            emv[:, u, :].unsqueeze(1).to_broadcast((128, 4, 4)), OP.add)
            sA.__exit__(None, None, None)

            # regroup A_c = Apre[:, L-1] to by-b layout [8, C*16]
            _sp2 = nc.named_scope("p2")
            _sp2.__enter__()
            abyb = singles.tile([BC, C * 16], F32)
            nc.sync.dma_start(abyb, Apre[:, L - 1, :, :].rearrange("p a b -> p (a b)"))
            abv = abyb.rearrange("p (c i j) -> p c i j", c=C, i=4)

            # ----- phase 2: boundary scores sbound[8, (C+1)*4], slot0 = 0 -----
            sbound = singles.tile([BC, (C + 1) * 4], F32)
            nc.vector.memset(sbound[:, 0:4], 0.0)
            sbv = sbound.rearrange("p (c j) -> p c j", c=C + 1)
            for c in range(C):
                p2tmp = tmp_pool.tile([BC, 4, 4], F32, tag="p2tmp")
                # tmp[j,i] = s[i] + A_c[i,j]
                nc.vector.tensor_tensor(
                    p2tmp,
                    sbv[:, c, :].unsqueeze(1).to_broadcast((BC, 4, 4)),
                    abv[:, c, :, :].transpose([0, 2, 1]),
                    OP.add)
                nc.vector.reduce_max(sbv[:, c + 1, :], p2tmp, axis=AX.X)
            _sp2.__exit__(None, None, None)

            # ----- phase 3 (parallel): scores[128, (L+1)*4] from boundary + Apre -----
            _sp3 = nc.named_scope("p3")
            _sp3.__enter__()
            scores = singles.tile([128, (L + 1) * 4], F32)
            nc.sync.dma_start(scores[:, 0:4], sbound[:, 0:C * 4])
            scv = scores.rearrange("p (u i) -> p u i", u=L + 1)
            p3tmp = singles.tile([128, L, 4, 4], F32)   # [u, j, i]
            nc.vector.tensor_tensor(
                p3tmp,
                scores[:, 0:4].unsqueeze(1).unsqueeze(1).to_broadcast((128, L, 4, 4)),
                Apre.transpose([0, 1, 3, 2]),
                OP.add)
            nc.vector.reduce_max(scv[:, 1:, :], p3tmp, axis=AX.X)
            _sp3.__exit__(None, None, None)

            # ----- phase 4: backpointer one-hots Pall[128, L, j, i] -----
            _sp4 = nc.named_scope("p4")
            _sp4.__enter__()
            cand = singles.tile([128, L, 4, 4], F32)
            nc.vector.tensor_tensor(
                cand,
                scv[:, 0:L, :].unsqueeze(2).to_broadcast((128, L, 4, 4)),
                ttr_ji.unsqueeze(1).to_broadcast((128, L, 4, 4)),
                OP.add)
            mxP = tmp_pool.tile([128, L, 4], F32, tag="mxP")
            nc.vector.reduce_max(mxP, cand, axis=AX.X)
            eqP = singles.tile([128, L, 4, 4], F32)
            nc.vector.tensor_tensor(eqP, cand, mxP.unsqueeze(3).to_broadcast((128, L, 4, 4)), OP.is_equal)
            nc.vector.tensor_tensor(
                eqP, eqP,
                wfirst.unsqueeze(1).unsqueeze(1).to_broadcast((128, L, 4, 4)),
                OP.mult)
            nc.vector.reduce_max(mxP, eqP, axis=AX.X)
            Pall = singles.tile([128, L, 4, 4], F32)
            nc.vector.tensor_tensor(Pall, eqP, mxP.unsqueeze(3).to_broadcast((128, L, 4, 4)), OP.is_equal)
            _sp4.__exit__(None, None, None)

            # ----- best_last one-hot on rows 0..7 -----
            ebyb = singles.tile([BC, C * 4], F32)
            ebv = ebyb.rearrange("p (c j) -> p c j", c=C)
            fin = tmp_pool.tile([BC, 4], F32, tag="fin")
            nc.vector.tensor_add(fin, sbv[:, C, :], end_sb[0:BC, :])
            mxf = tmp_pool.tile([BC, 1], F32, tag="mxf")
            nc.vector.reduce_max(mxf, fin, axis=AX.X)
            eqf = tmp_pool.tile([BC, 4], F32, tag="eqf")
            nc.vector.tensor_tensor(eqf, fin, mxf.to_broadcast((BC, 4)), OP.is_equal)
            nc.vector.tensor_tensor(eqf, eqf, wfirst[0:BC, :], OP.mult)
            nc.vector.reduce_max(mxf, eqf, axis=AX.X)
            nc.vector.tensor_tensor(ebv[:, C - 1, :], eqf, mxf.to_broadcast((BC, 4)), OP.is_equal)

            # ----- phase 5b: suffix maps Sall[128, L, x, i] + Ofull -----
            _sp5b = nc.named_scope("p5b")
            _sp5b.__enter__()
            Sall = singles.tile([128, L, 4, 4], F32)
            nc.vector.tensor_copy(Sall[:, L - 1, :, :], id4.rearrange("p (x i) -> p x i", x=4))
            for u in range(L - 2, -2, -1):
                p5tmp = tmp_pool.tile([128, 4, 4, 4], F32, tag="p5tmp")
                # tmp[x,i,y] = S_{u+1}[x,y] * P_{u+1}[y,i]
                nc.vector.tensor_tensor(
                    p5tmp,
                    Sall[:, u + 1, :, :].unsqueeze(2).to_broadcast((128, 4, 4, 4)),
                    Pall[:, u + 1, :, :].transpose([0, 2, 1]).unsqueeze(1).to_broadcast((128, 4, 4, 4)),
                    OP.mult)
                if u >= 0:
                    nc.vector.reduce_sum(Sall[:, u, :, :], p5tmp, axis=AX.X)
                else:
                    Ofull = singles.tile([128, 16], F32)
                    nc.vector.reduce_sum(Ofull.rearrange("p (x i) -> p x i", x=4),
                                         p5tmp, axis=AX.X)
            _sp5b.__exit__(None, None, None)

            # regroup Ofull to by-b [8, C*16]
            _sp5c = nc.named_scope("p5c")
            _sp5c.__enter__()
            obyb = singles.tile([BC, C * 16], F32)
            nc.sync.dma_start(obyb, Ofull)
            obv = obyb.rearrange("p (c x i) -> p c x i", c=C, x=4)

            # ----- phase 5c: boundary tags backward -----
            for c in range(C - 1, 0, -1):
                p5ctmp = tmp_pool.tile([BC, 4, 4], F32, tag="p5ctmp")
                # tmp[i,x] = E_c[x] * Ofull_c[x,i]
                nc.vector.tensor_tensor(
                    p5ctmp,
                    ebv[:, c, :].unsqueeze(1).to_broadcast((BC, 4, 4)),
                    obv[:, c, :, :].transpose([0, 2, 1]),
                    OP.mult)
                nc.vector.reduce_sum(ebv[:, c - 1, :], p5ctmp, axis=AX.X)

            # broadcast E to rows: ebc[128, 4], row b*16+c = E_c[b]
            ebc = singles.tile([128, 4], F32)
            nc.sync.dma_start(ebc, ebyb)
            _sp5c.__exit__(None, None, None)

            # ----- phase 5d: tags -----
            _sp5d = nc.named_scope("p5d")
            _sp5d.__enter__()
            G = tmp_pool.tile([128, 4, 4], F32, tag="G")
            nc.vector.tensor_tensor(
                G,
                ebc.unsqueeze(2).to_broadcast((128, 4, 4)),
                iw4.unsqueeze(1).to_broadcast((128, 4, 4)),
                OP.mult)
            p5dtmp = singles.tile([128, L, 4, 4], F32)
            nc.vector.tensor_tensor(
                p5dtmp, Sall,
                G.unsqueeze(1).to_broadcast((128, L, 4, 4)),
                OP.mult)
            tagf = tmp_pool.tile([128, L], F32, tag="tagf")
            nc.vector.reduce_sum(tagf, p5dtmp.rearrange("p u x i -> p u (x i)"), axis=AX.X)
            tagi = tmp_pool.tile([128, L], I32, tag="tagi")
            nc.vector.tensor_copy(tagi, tagf)
            nc.sync.dma_start(tagsd[:].rearrange("b (c t) -> b c t", c=C), tagi)
            _sp5d.__exit__(None, None, None)

    return nc


def _split_multi_waits(nc):
    """Walrus (bass2jax path) allows very few embedded sync waits per
    instruction (PE matmul: exactly 1). Hoist multi-waits onto standalone
    single-wait InstDrain instructions on the same engine, preserving order."""
    for f in nc.m.functions:
        for blk in f.blocks:
            insts = blk.instructions
            i = 0
            while i < len(insts):
                ins = insts[i]
                si = ins.sync_info
                w = list(si.on_wait) if (si is not None and si.on_wait) else []
                if len(w) >= 2:
                    for k, wait in enumerate(w):
                        d = mybir.InstDrain(
                            name=nc.get_next_instruction_name(),
                            ins=[], outs=[], bass_is_fusable=False)
                        d.engine = ins.engine
                        d.sync_info = mybir.SyncInfo(on_wait=[wait], on_update=[])
                        insts.insert(i + k, d)
                    i += len(w)
                    ins.sync_info = mybir.SyncInfo(
                        on_wait=[], on_update=list(si.on_update or []))
                i += 1


def _get_nc():
    if "nc" not in _NC_CACHE:
        nc = build_nc()
        _split_multi_waits(nc)   # HW path only; CoreSim rejects raw drains
        _NC_CACHE["nc"] = nc
    return _NC_CACHE["nc"]


def _consts():
    c = np.zeros(128 * 128 + 24, dtype=np.float32)
    c[:128 * 128] = np.eye(128, dtype=np.float32).ravel()
    c[128 * 128:128 * 128 + 4] = [4.0, 3.0, 2.0, 1.0]       # wfirst (4-i)
    c[128 * 128 + 4:128 * 128 + 8] = [0.0, 1.0, 2.0, 3.0]   # iw
    c[128 * 128 + 8:128 * 128 + 24] = np.eye(4, dtype=np.float32).ravel()
    return c


def make_in_maps(inputs):
    sent = np.ascontiguousarray(np.asarray(inputs["sentences"], dtype=np.float32))
    W = np.ascontiguousarray(np.asarray(inputs["W"], dtype=np.float32))
    bb = np.ascontiguousarray(np.asarray(inputs["b"], dtype=np.float32))
    st = np.ascontiguousarray(np.asarray(inputs["start_transitions"], dtype=np.float32))
    en = np.ascontiguousarray(np.asarray(inputs["end_transitions"], dtype=np.float32))
    tr = np.ascontiguousarray(np.asarray(inputs["transitions"], dtype=np.float32))
    consts = _consts()
    tinit = np.tile(tr.ravel(), (128, 1)).astype(np.float32)
    tinit[0::C, :] = np.tile(st, 4)[None, :]
    return [{
        "sentences": sent[c * BC:(c + 1) * BC],
        "W": W, "b": bb, "start_transitions": st,
        "end_transitions": en, "transitions": tr, "consts": consts,
        "tinit": tinit,
    } for c in range(NCORES)]


def kernel(**inputs):
    nc = _get_nc()
    in_maps = make_in_maps(inputs)
    res = run_bass_kernel_spmd(nc, in_maps, core_ids=list(range(NCORES)))
    tags = np.concatenate([res.results[c]["tags"] for c in range(NCORES)], axis=0)
    return tags.astype(np.int32)


if __name__ == "__main__":
    import reference
    inputs = {k: np.asarray(v) for k, v in reference.setup_inputs().items()}
    out = kernel(**inputs)
    print(out.shape, out.dtype, out[:2, :16])


# revision 9
# speedup vs baseline: 1.1539x; 1.1539x over previous
"""BERT-CRF Viterbi decode kernel for Trainium2 (Bass/Tile), 8-core data parallel.

Full inputs in, full outputs out. Internally shards batch B=64 across 8 cores
(8 sequences each). Per core, with scan rows r = b*16 + c (c = chunk of 32
timesteps):

  Stage A (u-tiled, fused with scan phase 1):
    for each scan step u (0..31), load sentences for all 128 rows at local
    step u, transpose h-chunks on PE, batched matmul (lhsT = W^T chunk [128,4],
    rhs = 4 steps' transposed sentences [128,512]) -> emissions^T in PSUM,
    fix-transpose back to [rows, 4], write directly into the SBUF scan tile.
    Phase 1 (chunk transfer-matrix recurrence) consumes each step's emissions
    as they land, hidden under stage A's PE/DMA time.
  Phase 2: boundary scores across chunks (sequential over 16, rows 0..7).
  Phase 3: all scores from boundaries + stored prefix matrices (2 big ops).
  Phase 4: backpointer one-hots, first-argmax semantics (6 big ops).
  Phase 5: one-hot matrix backtracking (no gathers).
"""
import sys
for p in ("/opt/trn_rl_repo", "/root/.axon_site/_ro/trn_rl_repo"):
    if p not in sys.path:
        sys.path.append(p)

import numpy as np
import concourse.bass as bass
import concourse.tile as tile
from concourse import mybir
from concourse.bass_utils import run_bass_kernel_spmd

F32 = mybir.dt.float32
I32 = mybir.dt.int32
AX = mybir.AxisListType
OP = mybir.AluOpType

B, T, H, K = 64, 512, 768, 4
NCORES = 8
BC = B // NCORES          # 8 sequences per core
C, L = 16, 32             # chunks per sequence, steps per chunk
ROWS = BC * C             # 128 partition rows
HCH = H // 128            # 6 h-chunks
UG = 4                    # steps per u-group (batched matmul width 4*128=512)

_NC_CACHE = {}


def build_nc():
    nc = bass.Bass()
    sent = nc.declare_dram_parameter("sentences", [BC, T, H], F32, isOutput=False)
    Wd = nc.declare_dram_parameter("W", [K, H], F32, isOutput=False)
    bd = nc.declare_dram_parameter("b", [K], F32, isOutput=False)
    startd = nc.declare_dram_parameter("start_transitions", [K], F32, isOutput=False)
    endd = nc.declare_dram_parameter("end_transitions", [K], F32, isOutput=False)
    transd = nc.declare_dram_parameter("transitions", [K, K], F32, isOutput=False)
    # consts: identity128 (128*128) ++ wfirst4 [4,3,2,1] ++ iw4 [0,1,2,3] ++ ident4 (16)
    constsd = nc.declare_dram_parameter("consts", [128 * 128 + 24], F32, isOutput=False)
    tinitd = nc.declare_dram_parameter("tinit", [128, 16], F32, isOutput=False)
    tagsd = nc.declare_dram_parameter("tags", [BC, T], I32, isOutput=True)

    with tile.TileContext(nc) as tc:
        with tc.tile_pool(name="singles", bufs=1) as singles, \
             tc.tile_pool(name="sent_pool", bufs=6) as sent_pool, \
             tc.tile_pool(name="st_pool", bufs=2) as st_pool, \
             tc.tile_pool(name="tmp_pool", bufs=2) as tmp_pool, \
             tc.tile_pool(name="ps_tr", bufs=4, space="PSUM") as ps_tr, \
             tc.tile_pool(name="ps_eT", bufs=2, space="PSUM") as ps_eT, \
             tc.tile_pool(name="ps_fix", bufs=2, space="PSUM") as ps_fix:

            # ---------- constants ----------
            ident = singles.tile([128, 128], F32)
            nc.sync.dma_start(ident, constsd[:][0:128 * 128].rearrange("(p f) -> p f", p=128))
            wfirst = singles.tile([128, 4], F32)
            nc.sync.dma_start(wfirst, constsd[:][128 * 128:128 * 128 + 4][None, :].to_broadcast((128, 4)))
            iw4 = singles.tile([128, 4], F32)
            nc.sync.dma_start(iw4, constsd[:][128 * 128 + 4:128 * 128 + 8][None, :].to_broadcast((128, 4)))
            id4 = singles.tile([128, 16], F32)
            nc.sync.dma_start(id4, constsd[:][128 * 128 + 8:128 * 128 + 24][None, :].to_broadcast((128, 16)))
            end_sb = singles.tile([128, 4], F32)
            nc.sync.dma_start(end_sb, endd[:][None, :].to_broadcast((128, 4)))
            ttr = singles.tile([128, 16], F32)
            nc.sync.dma_start(ttr, transd[:].rearrange("i j -> (i j)")[None, :].to_broadcast((128, 16)))
            tinit = singles.tile([128, 16], F32)
            nc.sync.dma_start(tinit, tinitd[:])
            b_sb = singles.tile([1, 4], F32)
            nc.sync.dma_start(b_sb, bd[:][None, :])
            ones_sb = singles.tile([1, UG * 128], F32)
            nc.vector.memset(ones_sb, 1.0)

            # ---------- W^T in SBUF: wt[p = h within chunk, ch, k] ----------
            w_raw = singles.tile([K, H], F32)
            nc.sync.dma_start(w_raw, Wd[:])
            wt_sb = singles.tile([128, HCH, K], F32)
            for ch in range(HCH):
                wt_ps = ps_fix.tile([128, K], F32, tag="fix")
                nc.tensor.transpose(wt_ps, w_raw[:, ch * 128:(ch + 1) * 128], ident[0:K, 0:K])
                nc.scalar.copy(wt_sb[:, ch, :], wt_ps)

            # scan emissions tile, written directly by stage A
            emsc = singles.tile([128, L * K], F32)
            emv = emsc.rearrange("p (u j) -> p u j", u=L)

            # phase-1 state: prefix transfer matrices Apre[row, u, i, j]
            Apre = singles.tile([128, L, 4, 4], F32)

            # views
            ttrT_v = ttr.rearrange("p (k j) -> p k j", k=4).transpose([0, 2, 1])  # [p,j,k] = trans[k,j]
            ttr_ji = ttr.rearrange("p (i j) -> p i j", i=4).transpose([0, 2, 1])  # [p,j,i] = trans[i,j]

            # ---------- Stage A (u-tiled) fused with phase 1 ----------
            sA = nc.named_scope("stageA")
            sA.__enter__()
            for g in range(L // UG):
                sents = []
                for uu in range(UG):
                    u = g * UG + uu
                    s_sb = sent_pool.tile([128, H], F32)
                    # row (b*16+c) <- sentences[b, c*32 + u, :]
                    src = bass.AP(
                        tensor=sent[:].tensor, offset=u * H,
                        ap=[[T * H, BC], [L * H, C], [1, H]])
                    nc.sync.dma_start(s_sb, src)
                    sents.append(s_sb)
                # transposes: sT[p=h, ch, uu, rows]
                sT_sb = st_pool.tile([128, HCH, UG, 128], F32)
                for ch in range(HCH):
                    for uu in range(0, UG, 2):
                        trp = ps_tr.tile([128, 256], F32, tag="trps")
                        nc.tensor.transpose(
                            trp[:, 0:128], sents[uu][:, ch * 128:(ch + 1) * 128], ident)
                        nc.tensor.transpose(
                            trp[:, 128:256], sents[uu + 1][:, ch * 128:(ch + 1) * 128], ident)
                        nc.scalar.copy(
                            sT_sb[:, ch, uu:uu + 2, :].rearrange("p a b -> p (a b)"), trp)
                # batched matmuls: out eT[k, uu*128+row] accum over ch, + bias
                eT_ps = ps_eT.tile([4, UG * 128], F32, tag="eT")
                for ch in range(HCH):
                    nc.tensor.matmul(
                        eT_ps, wt_sb[:, ch, :],
                        sT_sb[:, ch, :, :].rearrange("p a b -> p (a b)"),
                        start=(ch == 0), stop=False)
                nc.tensor.matmul(eT_ps, b_sb, ones_sb, start=False, stop=True)
                eT_sb = st_pool.tile([4, UG * 128], F32, tag="eTsb")
                nc.scalar.copy(eT_sb, eT_ps)
                # fix-transpose each uu back to [rows, 4] and land in emsc
                for uu in range(UG):
                    u = g * UG + uu
                    fx = ps_fix.tile([128, K], F32, tag="fix")
                    nc.tensor.transpose(
                        fx, eT_sb[:, uu * 128:(uu + 1) * 128], ident[0:K, 0:K])
                    nc.scalar.copy(emsc[:, u * 4:(u + 1) * 4], fx)
                # ---- phase 1 steps for this group ----
                for uu in range(UG):
                    u = g * UG + uu
                    if u == 0:
                        nc.vector.tensor_tensor(
                            Apre[:, 0, :, :],
                            tinit.rearrange("p (i j) -> p i j", i=4),
                            emv[:, 0, :].unsqueeze(1).to_broadcast((128, 4, 4)),
                            OP.add)
                    else:
                        p1tmp = tmp_pool.tile([128, 4, 4, 4], F32, tag="p1tmp")
                        # tmp[i,j,k] = A[i,k] + trans[k,j]
                        nc.vector.tensor_tensor(
                            p1tmp,
                            Apre[:, u - 1, :, :].unsqueeze(2).to_broadcast((128, 4, 4, 4)),
                            ttrT_v.unsqueeze(1).to_broadcast((128, 4, 4, 4)),
                            OP.add)
                        p1red = tmp_pool.tile([128, 4, 4], F32, tag="p1red")
                        nc.vector.reduce_max(p1red, p1tmp, axis=AX.X)
                        nc.vector.tensor_tensor(
                            Apre[:, u, :, :], p1red,
                            emv[:, u, :].unsqueeze(1).to_broadcast((128, 4, 4)), OP.add)
            sA.__exit__(None, None, None)

            # regroup A_c = Apre[:, L-1] to by-b layout [8, C*16]
            _sp2 = nc.named_scope("p2")
            _sp2.__enter__()
            abyb = singles.tile([BC, C * 16], F32)
            nc.sync.dma_start(abyb, Apre[:, L - 1, :, :].rearrange("p a b -> p (a b)"))
            abv = abyb.rearrange("p (c i j) -> p c i j", c=C, i=4)

            # ----- phase 2: boundary scores sbound[8, (C+1)*4], slot0 = 0 -----
            sbound = singles.tile([BC, (C + 1) * 4], F32)
            nc.vector.memset(sbound[:, 0:4], 0.0)
            sbv = sbound.rearrange("p (c j) -> p c j", c=C + 1)
            for c in range(C):
                p2tmp = tmp_pool.tile([BC, 4, 4], F32, tag="p2tmp")
                # tmp[j,i] = s[i] + A_c[i,j]
                nc.vector.tensor_tensor(
                    p2tmp,
                    sbv[:, c, :].unsqueeze(1).to_broadcast((BC, 4, 4)),
                    abv[:, c, :, :].transpose([0, 2, 1]),
                    OP.add)
                nc.vector.reduce_max(sbv[:, c + 1, :], p2tmp, axis=AX.X)
            _sp2.__exit__(None, None, None)

            # ----- phase 3 (parallel): scores[128, (L+1)*4] from boundary + Apre -----
            _sp3 = nc.named_scope("p3")
            _sp3.__enter__()
            scores = singles.tile([128, (L + 1) * 4], F32)
            nc.sync.dma_start(scores[:, 0:4], sbound[:, 0:C * 4])
            scv = scores.rearrange("p (u i) -> p u i", u=L + 1)
            p3tmp = singles.tile([128, L, 4, 4], F32)   # [u, j, i]
            nc.vector.tensor_tensor(
                p3tmp,
                scores[:, 0:4].unsqueeze(1).unsqueeze(1).to_broadcast((128, L, 4, 4)),
                Apre.transpose([0, 1, 3, 2]),
                OP.add)
            nc.vector.reduce_max(scv[:, 1:, :], p3tmp, axis=AX.X)
            _sp3.__exit__(None, None, None)

            # ----- phase 4: backpointer one-hots Pall[128, L, j, i] -----
            _sp4 = nc.named_scope("p4")
            _sp4.__enter__()
            cand = singles.tile([128, L, 4, 4], F32)
            nc.vector.tensor_tensor(
                cand,
                scv[:, 0:L, :].unsqueeze(2).to_broadcast((128, L, 4, 4)),
                ttr_ji.unsqueeze(1).to_broadcast((128, L, 4, 4)),
                OP.add)
            mxP = tmp_pool.tile([128, L, 4], F32, tag="mxP")
            nc.vector.reduce_max(mxP, cand, axis=AX.X)
            eqP = singles.tile([128, L, 4, 4], F32)
            nc.vector.tensor_tensor(eqP, cand, mxP.unsqueeze(3).to_broadcast((128, L, 4, 4)), OP.is_equal)
            nc.vector.tensor_tensor(
                eqP, eqP,
                wfirst.unsqueeze(1).unsqueeze(1).to_broadcast((128, L, 4, 4)),
                OP.mult)
            nc.vector.reduce_max(mxP, eqP, axis=AX.X)
            Pall = singles.tile([128, L, 4, 4], F32)
            nc.vector.tensor_tensor(Pall, eqP, mxP.unsqueeze(3).to_broadcast((128, L, 4, 4)), OP.is_equal)
            _sp4.__exit__(None, None, None)

            # ----- best_last one-hot on rows 0..7 -----
            ebyb = singles.tile([BC, C * 4], F32)
            ebv = ebyb.rearrange("p (c j) -> p c j", c=C)
            fin = tmp_pool.tile([BC, 4], F32, tag="fin")
            nc.vector.tensor_add(fin, sbv[:, C, :], end_sb[0:BC, :])
            mxf = tmp_pool.tile([BC, 1], F32, tag="mxf")
            nc.vector.reduce_max(mxf, fin, axis=AX.X)
            eqf = tmp_pool.tile([BC, 4], F32, tag="eqf")
            nc.vector.tensor_tensor(eqf, fin, mxf.to_broadcast((BC, 4)), OP.is_equal)
            nc.vector.tensor_tensor(eqf, eqf, wfirst[0:BC, :], OP.mult)
            nc.vector.reduce_max(mxf, eqf, axis=AX.X)
            nc.vector.tensor_tensor(ebv[:, C - 1, :], eqf, mxf.to_broadcast((BC, 4)), OP.is_equal)

            # ----- phase 5b: suffix maps Sall[128, L, x, i] + Ofull -----
            _sp5b = nc.named_scope("p5b")
            _sp5b.__enter__()
            Sall = singles.tile([128, L, 4, 4], F32)
            nc.vector.tensor_copy(Sall[:, L - 1, :, :], id4.rearrange("p (x i) -> p x i", x=4))
            for u in range(L - 2, -2, -1):
                p5tmp = tmp_pool.tile([128, 4, 4, 4], F32, tag="p5tmp")
                # tmp[x,i,y] = S_{u+1}[x,y] * P_{u+1}[y,i]
                nc.vector.tensor_tensor(
                    p5tmp,
                    Sall[:, u + 1, :, :].unsqueeze(2).to_broadcast((128, 4, 4, 4)),
                    Pall[:, u + 1, :, :].transpose([0, 2, 1]).unsqueeze(1).to_broadcast((128, 4, 4, 4)),
                    OP.mult)
                if u >= 0:
                    nc.vector.reduce_sum(Sall[:, u, :, :], p5tmp, axis=AX.X)
                else:
                    Ofull = singles.tile([128, 16], F32)
                    nc.vector.reduce_sum(Ofull.rearrange("p (x i) -> p x i", x=4),
                                         p5tmp, axis=AX.X)
            _sp5b.__exit__(None, None, None)

            # regroup Ofull to by-b [8, C*16]
            _sp5c = nc.named_scope("p5c")
            _sp5c.__enter__()
            obyb = singles.tile([BC, C * 16], F32)
            nc.sync.dma_start(obyb, Ofull)
            obv = obyb.rearrange("p (c x i) -> p c x i", c=C, x=4)

            # ----- phase 5c: boundary tags backward -----
            for c in range(C - 1, 0, -1):
                p5ctmp = tmp_pool.tile([BC, 4, 4], F32, tag="p5ctmp")
                # tmp[i,x] = E_c[x] * Ofull_c[x,i]
                nc.vector.tensor_tensor(
                    p5ctmp,
                    ebv[:, c, :].unsqueeze(1).to_broadcast((BC, 4, 4)),
                    obv[:, c, :, :].transpose([0, 2, 1]),
                    OP.mult)
                nc.vector.reduce_sum(ebv[:, c - 1, :], p5ctmp, axis=AX.X)

            # broadcast E to rows: ebc[128, 4], row b*16+c = E_c[b]
            ebc = singles.tile([128, 4], F32)
            nc.sync.dma_start(ebc, ebyb)
            _sp5c.__exit__(None, None, None)

            # ----- phase 5d: tags -----
            _sp5d = nc.named_scope("p5d")
            _sp5d.__enter__()
            G = tmp_pool.tile([128, 4, 4], F32, tag="G")
            nc.vector.tensor_tensor(
                G,
                ebc.unsqueeze(2).to_broadcast((128, 4, 4)),
                iw4.unsqueeze(1).to_broadcast((128, 4, 4)),
                OP.mult)
            p5dtmp = singles.tile([128, L, 4, 4], F32)
            nc.vector.tensor_tensor(
                p5dtmp, Sall,
                G.unsqueeze(1).to_broadcast((128, L, 4, 4)),
                OP.mult)
            tagf = tmp_pool.tile([128, L], F32, tag="tagf")
            nc.vector.reduce_sum(tagf, p5dtmp.rearrange("p u x i -> p u (x i)"), axis=AX.X)
            tagi = tmp_pool.tile([128, L], I32, tag="tagi")
            nc.vector.tensor_copy(tagi, tagf)
            nc.sync.dma_start(tagsd[:].rearrange("b (c t) -> b c t", c=C), tagi)
            _sp5d.__exit__(None, None, None)

    return nc


def _split_multi_waits(nc):
    """Walrus (bass2jax path) allows very few embedded sync waits per
    instruction (PE matmul: exactly 1). Hoist multi-waits onto standalone
    single-wait InstDrain instructions on the same engine, preserving order."""
    for f in nc.m.functions:
        for blk in f.blocks:
            insts = blk.instructions
            i = 0
            while i < len(insts):
                ins = insts[i]
                si = ins.sync_info
                w = list(si.on_wait) if (si is not None and si.on_wait) else []
                if len(w) >= 2:
                    for k, wait in enumerate(w):
                        d = mybir.InstEventSemaphore(
                            name=nc.get_next_instruction_name(), ins=[], outs=[])
                        d.engine = ins.engine
                        d.sync_info = mybir.SyncInfo(on_wait=[wait], on_update=[])
                        insts.insert(i + k, d)
                    i += len(w)
                    ins.sync_info = mybir.SyncInfo(
                        on_wait=[], on_update=list(si.on_update or []))
                i += 1


def _get_nc():
    if "nc" not in _NC_CACHE:
        nc = build_nc()
        _split_multi_waits(nc)   # HW path only; CoreSim rejects raw drains
        _NC_CACHE["nc"] = nc
    return _NC_CACHE["nc"]


def _consts():
    c = np.zeros(128 * 128 + 24, dtype=np.float32)
    c[:128 * 128] = np.eye(128, dtype=np.float32).ravel()
    c[128 * 128:128 * 128 + 4] = [4.0, 3.0, 2.0, 1.0]       # wfirst (4-i)
    c[128 * 128 + 4:128 * 128 + 8] = [0.0, 1.0, 2.0, 3.0]   # iw
    c[128 * 128 + 8:128 * 128 + 24] = np.eye(4, dtype=np.float32).ravel()
    return c


def make_in_maps(inputs):
    sent = np.ascontiguousarray(np.asarray(inputs["sentences"], dtype=np.float32))
    W = np.ascontiguousarray(np.asarray(inputs["W"], dtype=np.float32))
    bb = np.ascontiguousarray(np.asarray(inputs["b"], dtype=np.float32))
    st = np.ascontiguousarray(np.asarray(inputs["start_transitions"], dtype=np.float32))
    en = np.ascontiguousarray(np.asarray(inputs["end_transitions"], dtype=np.float32))
    tr = np.ascontiguousarray(np.asarray(inputs["transitions"], dtype=np.float32))
    consts = _consts()
    tinit = np.tile(tr.ravel(), (128, 1)).astype(np.float32)
    tinit[0::C, :] = np.tile(st, 4)[None, :]
    return [{
        "sentences": sent[c * BC:(c + 1) * BC],
        "W": W, "b": bb, "start_transitions": st,
        "end_transitions": en, "transitions": tr, "consts": consts,
        "tinit": tinit,
    } for c in range(NCORES)]


def kernel(**inputs):
    nc = _get_nc()
    in_maps = make_in_maps(inputs)
    res = run_bass_kernel_spmd(nc, in_maps, core_ids=list(range(NCORES)))
    tags = np.concatenate([res.results[c]["tags"] for c in range(NCORES)], axis=0)
    return tags.astype(np.int32)


if __name__ == "__main__":
    import reference
    inputs = {k: np.asarray(v) for k, v in reference.setup_inputs().items()}
    out = kernel(**inputs)
    print(out.shape, out.dtype, out[:2, :16])


# revision 10
# speedup vs baseline: 1.2270x; 1.0634x over previous
"""BERT-CRF Viterbi decode kernel for Trainium2 (Bass/Tile), 8-core data parallel.

Full inputs in, full outputs out. Internally shards batch B=64 across 8 cores
(8 sequences each). Per core, with scan rows r = b*16 + c (c = chunk of 32
timesteps):

  Stage A (u-tiled, fused with scan phase 1):
    for each scan step u (0..31), load sentences for all 128 rows at local
    step u, transpose h-chunks on PE, batched matmul (lhsT = W^T chunk [128,4],
    rhs = 4 steps' transposed sentences [128,512]) -> emissions^T in PSUM,
    fix-transpose back to [rows, 4], write directly into the SBUF scan tile.
    Phase 1 (chunk transfer-matrix recurrence) consumes each step's emissions
    as they land, hidden under stage A's PE/DMA time.
  Phase 2: boundary scores across chunks (sequential over 16, rows 0..7).
  Phase 3: all scores from boundaries + stored prefix matrices (2 big ops).
  Phase 4: backpointer one-hots, first-argmax semantics (6 big ops).
  Phase 5: one-hot matrix backtracking (no gathers).
"""
import sys
for p in ("/opt/trn_rl_repo", "/root/.axon_site/_ro/trn_rl_repo"):
    if p not in sys.path:
        sys.path.append(p)

import numpy as np
import concourse.bass as bass
import concourse.tile as tile
from concourse import mybir
from concourse.bass_utils import run_bass_kernel_spmd

F32 = mybir.dt.float32
I32 = mybir.dt.int32
AX = mybir.AxisListType
OP = mybir.AluOpType

B, T, H, K = 64, 512, 768, 4
NCORES = 8
BC = B // NCORES          # 8 sequences per core
C, L = 16, 32             # chunks per sequence, steps per chunk
ROWS = BC * C             # 128 partition rows
HCH = H // 128            # 6 h-chunks
UG = 4                    # steps per u-group (batched matmul width 4*128=512)

_NC_CACHE = {}


def build_nc():
    nc = bass.Bass()
    sent = nc.declare_dram_parameter("sentences", [BC, T, H], F32, isOutput=False)
    Wd = nc.declare_dram_parameter("W", [K, H], F32, isOutput=False)
    identd = nc.declare_dram_parameter("identc", [128, 128], F32, isOutput=False)
    # rowconsts[128, 64]: wfirst | iw | ident4 | end | ttr | tinit | bias
    rcd = nc.declare_dram_parameter("rowconsts", [128, 64], F32, isOutput=False)
    tagsd = nc.declare_dram_parameter("tags", [BC, T], I32, isOutput=True)

    with tile.TileContext(nc) as tc:
        with tc.tile_pool(name="singles", bufs=1) as singles, \
             tc.tile_pool(name="sent_pool", bufs=6) as sent_pool, \
             tc.tile_pool(name="st_pool", bufs=2) as st_pool, \
             tc.tile_pool(name="tmp_pool", bufs=2) as tmp_pool, \
             tc.tile_pool(name="ps_tr", bufs=3, space="PSUM") as ps_tr, \
             tc.tile_pool(name="ps_eT", bufs=2, space="PSUM") as ps_eT, \
             tc.tile_pool(name="ps_fix", bufs=2, space="PSUM") as ps_fix:

            # ---------- prefetch first sentence group ----------
            pre_sents = []
            for uu in range(UG):
                s_sb = sent_pool.tile([128, H], F32, tag="sent")
                src0 = bass.AP(
                    tensor=sent[:].tensor, offset=uu * H,
                    ap=[[T * H, BC], [L * H, C], [1, H]])
                nc.sync.dma_start(s_sb, src0)
                pre_sents.append(s_sb)

            # ---------- constants ----------
            ident = singles.tile([128, 128], F32)
            nc.sync.dma_start(ident, identd[:])
            rc = singles.tile([128, 64], F32)
            nc.sync.dma_start(rc, rcd[:])
            wfirst = rc[:, 0:4]
            iw4 = rc[:, 4:8]
            id4 = rc[:, 8:24]
            end_sb = rc[:, 24:28]
            ttr = rc[:, 28:44]
            tinit = rc[:, 44:60]
            bias4 = rc[:, 60:64]

            # ---------- W^T in SBUF: wt[p = h within chunk, ch, k] ----------
            w_raw = singles.tile([K, H], F32)
            nc.sync.dma_start(w_raw, Wd[:])
            wt_sb = singles.tile([128, HCH, K], F32)
            for ch in range(HCH):
                wt_ps = ps_fix.tile([128, K], F32, tag="fix")
                nc.tensor.transpose(wt_ps, w_raw[:, ch * 128:(ch + 1) * 128], ident[0:K, 0:K])
                nc.scalar.copy(wt_sb[:, ch, :], wt_ps)

            # scan emissions tile, written directly by stage A
            emsc = singles.tile([128, L * K], F32)
            emv = emsc.rearrange("p (u j) -> p u j", u=L)

            # phase-1 state: prefix transfer matrices Apre[row, u, i, j]
            Apre = singles.tile([128, L, 4, 4], F32)

            # views
            ttrT_v = ttr.rearrange("p (k j) -> p k j", k=4).transpose([0, 2, 1])  # [p,j,k] = trans[k,j]
            ttr_ji = ttr.rearrange("p (i j) -> p i j", i=4).transpose([0, 2, 1])  # [p,j,i] = trans[i,j]

            # ---------- Stage A (u-tiled) fused with phase 1 ----------
            sA = nc.named_scope("stageA")
            sA.__enter__()
            for g in range(L // UG):
                if g == 0:
                    sents = pre_sents
                else:
                    sents = []
                    for uu in range(UG):
                        u = g * UG + uu
                        s_sb = sent_pool.tile([128, H], F32, tag="sent")
                        # row (b*16+c) <- sentences[b, c*32 + u, :]
                        src = bass.AP(
                            tensor=sent[:].tensor, offset=u * H,
                            ap=[[T * H, BC], [L * H, C], [1, H]])
                        nc.sync.dma_start(s_sb, src)
                        sents.append(s_sb)
                # transposes: sT[p=h, ch, uu, rows]
                sT_sb = st_pool.tile([128, HCH, UG, 128], F32)
                for ch in range(HCH):
                    trp = ps_tr.tile([128, UG * 128], F32, tag="trps")
                    for uu in range(UG):
                        nc.tensor.transpose(
                            trp[:, uu * 128:(uu + 1) * 128],
                            sents[uu][:, ch * 128:(ch + 1) * 128], ident)
                    nc.scalar.copy(
                        sT_sb[:, ch, :, :].rearrange("p a b -> p (a b)"), trp)
                # batched matmuls: out eT[k, uu*128+row] accum over ch
                eT_ps = ps_eT.tile([4, UG * 128], F32, tag="eT")
                for ch in range(HCH):
                    nc.tensor.matmul(
                        eT_ps, wt_sb[:, ch, :],
                        sT_sb[:, ch, :, :].rearrange("p a b -> p (a b)"),
                        start=(ch == 0), stop=(ch == HCH - 1))
                eT_sb = st_pool.tile([4, UG * 128], F32, tag="eTsb")
                nc.scalar.copy(eT_sb, eT_ps)
                # fix-transpose each uu back to [rows, 4] and land in emsc
                for uu in range(UG):
                    u = g * UG + uu
                    fx = ps_fix.tile([128, K], F32, tag="fix")
                    nc.tensor.transpose(
                        fx, eT_sb[:, uu * 128:(uu + 1) * 128], ident[0:K, 0:K])
                    nc.scalar.copy(emsc[:, u * 4:(u + 1) * 4], fx)
                # bias (reference adds b last): emsc[:, g] += b
                nc.vector.tensor_tensor(
                    emv[:, g * UG:(g + 1) * UG, :],
                    emv[:, g * UG:(g + 1) * UG, :],
                    bias4.unsqueeze(1).to_broadcast((128, UG, 4)),
                    OP.add)
                # ---- phase 1 steps for this group ----
                for uu in range(UG):
                    u = g * UG + uu
                    if u == 0:
                        nc.vector.tensor_tensor(
                            Apre[:, 0, :, :],
                            tinit.rearrange("p (i j) -> p i j", i=4),
                            emv[:, 0, :].unsqueeze(1).to_broadcast((128, 4, 4)),
                            OP.add)
                    else:
                        p1tmp = tmp_pool.tile([128, 4, 4, 4], F32, tag="p1tmp")
                        # tmp[i,j,k] = A[i,k] + trans[k,j]
                        nc.vector.tensor_tensor(
                            p1tmp,
                            Apre[:, u - 1, :, :].unsqueeze(2).to_broadcast((128, 4, 4, 4)),
                            ttrT_v.unsqueeze(1).to_broadcast((128, 4, 4, 4)),
                            OP.add)
                        p1red = tmp_pool.tile([128, 4, 4], F32, tag="p1red")
                        nc.vector.reduce_max(p1red, p1tmp, axis=AX.X)
                        nc.vector.tensor_tensor(
                            Apre[:, u, :, :], p1red,
                            emv[:, u, :].unsqueeze(1).to_broadcast((128, 4, 4)), OP.add)
            sA.__exit__(None, None, None)

            # regroup A_c = Apre[:, L-1] to by-b layout [8, C*16]
            _sp2 = nc.named_scope("p2")
            _sp2.__enter__()
            abyb = singles.tile([BC, C * 16], F32)
            nc.sync.dma_start(abyb, Apre[:, L - 1, :, :].rearrange("p a b -> p (a b)"))
            abv = abyb.rearrange("p (c i j) -> p c i j", c=C, i=4)

            # ----- phase 2: boundary scores sbound[8, (C+1)*4], slot0 = 0 -----
            sbound = singles.tile([BC, (C + 1) * 4], F32)
            nc.vector.memset(sbound[:, 0:4], 0.0)
            sbv = sbound.rearrange("p (c j) -> p c j", c=C + 1)
            for c in range(C):
                p2tmp = tmp_pool.tile([BC, 4, 4], F32, tag="p2tmp")
                # tmp[j,i] = s[i] + A_c[i,j]
                nc.vector.tensor_tensor(
                    p2tmp,
                    sbv[:, c, :].unsqueeze(1).to_broadcast((BC, 4, 4)),
                    abv[:, c, :, :].transpose([0, 2, 1]),
                    OP.add)
                nc.vector.reduce_max(sbv[:, c + 1, :], p2tmp, axis=AX.X)
            _sp2.__exit__(None, None, None)

            # ----- phase 3 (parallel): scores[128, (L+1)*4] from boundary + Apre -----
            _sp3 = nc.named_scope("p3")
            _sp3.__enter__()
            scores = singles.tile([128, (L + 1) * 4], F32)
            nc.sync.dma_start(scores[:, 0:4], sbound[:, 0:C * 4])
            scv = scores.rearrange("p (u i) -> p u i", u=L + 1)
            p3tmp = singles.tile([128, L, 4, 4], F32)   # [u, j, i]
            nc.vector.tensor_tensor(
                p3tmp,
                scores[:, 0:4].unsqueeze(1).unsqueeze(1).to_broadcast((128, L, 4, 4)),
                Apre.transpose([0, 1, 3, 2]),
                OP.add)
            nc.vector.reduce_max(scv[:, 1:, :], p3tmp, axis=AX.X)
            _sp3.__exit__(None, None, None)

            # ----- phase 4: backpointer one-hots Pall[128, L, j, i] -----
            _sp4 = nc.named_scope("p4")
            _sp4.__enter__()
            cand = singles.tile([128, L, 4, 4], F32)
            nc.vector.tensor_tensor(
                cand,
                scv[:, 0:L, :].unsqueeze(2).to_broadcast((128, L, 4, 4)),
                ttr_ji.unsqueeze(1).to_broadcast((128, L, 4, 4)),
                OP.add)
            mxP = tmp_pool.tile([128, L, 4], F32, tag="mxP")
            nc.vector.reduce_max(mxP, cand, axis=AX.X)
            eqP = singles.tile([128, L, 4, 4], F32)
            nc.vector.tensor_tensor(eqP, cand, mxP.unsqueeze(3).to_broadcast((128, L, 4, 4)), OP.is_equal)
            nc.vector.tensor_tensor(
                eqP, eqP,
                wfirst.unsqueeze(1).unsqueeze(1).to_broadcast((128, L, 4, 4)),
                OP.mult)
            nc.vector.reduce_max(mxP, eqP, axis=AX.X)
            Pall = singles.tile([128, L, 4, 4], F32)
            nc.vector.tensor_tensor(Pall, eqP, mxP.unsqueeze(3).to_broadcast((128, L, 4, 4)), OP.is_equal)
            _sp4.__exit__(None, None, None)

            # ----- best_last one-hot on rows 0..7 -----
            ebyb = singles.tile([BC, C * 4], F32)
            ebv = ebyb.rearrange("p (c j) -> p c j", c=C)
            fin = tmp_pool.tile([BC, 4], F32, tag="fin")
            nc.vector.tensor_add(fin, sbv[:, C, :], end_sb[0:BC, :])
            mxf = tmp_pool.tile([BC, 1], F32, tag="mxf")
            nc.vector.reduce_max(mxf, fin, axis=AX.X)
            eqf = tmp_pool.tile([BC, 4], F32, tag="eqf")
            nc.vector.tensor_tensor(eqf, fin, mxf.to_broadcast((BC, 4)), OP.is_equal)
            nc.vector.tensor_tensor(eqf, eqf, wfirst[0:BC, :], OP.mult)
            nc.vector.reduce_max(mxf, eqf, axis=AX.X)
            nc.vector.tensor_tensor(ebv[:, C - 1, :], eqf, mxf.to_broadcast((BC, 4)), OP.is_equal)

            # ----- phase 5b: suffix maps Sall[128, L, x, i] + Ofull -----
            _sp5b = nc.named_scope("p5b")
            _sp5b.__enter__()
            Sall = singles.tile([128, L, 4, 4], F32)
            nc.vector.tensor_copy(Sall[:, L - 1, :, :], id4.rearrange("p (x i) -> p x i", x=4))
            for u in range(L - 2, -2, -1):
                p5tmp = tmp_pool.tile([128, 4, 4, 4], F32, tag="p5tmp")
                # tmp[x,i,y] = S_{u+1}[x,y] * P_{u+1}[y,i]
                nc.vector.tensor_tensor(
                    p5tmp,
                    Sall[:, u + 1, :, :].unsqueeze(2).to_broadcast((128, 4, 4, 4)),
                    Pall[:, u + 1, :, :].transpose([0, 2, 1]).unsqueeze(1).to_broadcast((128, 4, 4, 4)),
                    OP.mult)
                if u >= 0:
                    nc.vector.reduce_sum(Sall[:, u, :, :], p5tmp, axis=AX.X)
                else:
                    Ofull = singles.tile([128, 16], F32)
                    nc.vector.reduce_sum(Ofull.rearrange("p (x i) -> p x i", x=4),
                                         p5tmp, axis=AX.X)
            _sp5b.__exit__(None, None, None)

            # regroup Ofull to by-b [8, C*16]
            _sp5c = nc.named_scope("p5c")
            _sp5c.__enter__()
            obyb = singles.tile([BC, C * 16], F32)
            nc.sync.dma_start(obyb, Ofull)
            obv = obyb.rearrange("p (c x i) -> p c x i", c=C, x=4)

            # ----- phase 5c: boundary tags backward -----
            for c in range(C - 1, 0, -1):
                p5ctmp = tmp_pool.tile([BC, 4, 4], F32, tag="p5ctmp")
                # tmp[i,x] = E_c[x] * Ofull_c[x,i]
                nc.vector.tensor_tensor(
                    p5ctmp,
                    ebv[:, c, :].unsqueeze(1).to_broadcast((BC, 4, 4)),
                    obv[:, c, :, :].transpose([0, 2, 1]),
                    OP.mult)
                nc.vector.reduce_sum(ebv[:, c - 1, :], p5ctmp, axis=AX.X)

            # broadcast E to rows: ebc[128, 4], row b*16+c = E_c[b]
            ebc = singles.tile([128, 4], F32)
            nc.sync.dma_start(ebc, ebyb)
            _sp5c.__exit__(None, None, None)

            # ----- phase 5d: tags -----
            _sp5d = nc.named_scope("p5d")
            _sp5d.__enter__()
            G = tmp_pool.tile([128, 4, 4], F32, tag="G")
            nc.vector.tensor_tensor(
                G,
                ebc.unsqueeze(2).to_broadcast((128, 4, 4)),
                iw4.unsqueeze(1).to_broadcast((128, 4, 4)),
                OP.mult)
            p5dtmp = singles.tile([128, L, 4, 4], F32)
            nc.vector.tensor_tensor(
                p5dtmp, Sall,
                G.unsqueeze(1).to_broadcast((128, L, 4, 4)),
                OP.mult)
            tagf = tmp_pool.tile([128, L], F32, tag="tagf")
            nc.vector.reduce_sum(tagf, p5dtmp.rearrange("p u x i -> p u (x i)"), axis=AX.X)
            tagi = tmp_pool.tile([128, L], I32, tag="tagi")
            nc.vector.tensor_copy(tagi, tagf)
            nc.sync.dma_start(tagsd[:].rearrange("b (c t) -> b c t", c=C), tagi)
            _sp5d.__exit__(None, None, None)

    return nc


def _split_multi_waits(nc):
    """Walrus (bass2jax path) allows very few embedded sync waits per
    instruction (PE matmul: exactly 1). Hoist multi-waits onto standalone
    single-wait InstDrain instructions on the same engine, preserving order."""
    for f in nc.m.functions:
        for blk in f.blocks:
            insts = blk.instructions
            i = 0
            while i < len(insts):
                ins = insts[i]
                si = ins.sync_info
                w = list(si.on_wait) if (si is not None and si.on_wait) else []
                if len(w) >= 2:
                    for k, wait in enumerate(w):
                        d = mybir.InstEventSemaphore(
                            name=nc.get_next_instruction_name(), ins=[], outs=[])
                        d.engine = ins.engine
                        d.sync_info = mybir.SyncInfo(on_wait=[wait], on_update=[])
                        insts.insert(i + k, d)
                    i += len(w)
                    ins.sync_info = mybir.SyncInfo(
                        on_wait=[], on_update=list(si.on_update or []))
                i += 1


def _get_nc():
    if "nc" not in _NC_CACHE:
        nc = build_nc()
        _split_multi_waits(nc)   # HW path only; CoreSim rejects raw drains
        _NC_CACHE["nc"] = nc
    return _NC_CACHE["nc"]


def make_in_maps(inputs):
    sent = np.ascontiguousarray(np.asarray(inputs["sentences"], dtype=np.float32))
    W = np.ascontiguousarray(np.asarray(inputs["W"], dtype=np.float32))
    bb = np.ascontiguousarray(np.asarray(inputs["b"], dtype=np.float32))
    st = np.ascontiguousarray(np.asarray(inputs["start_transitions"], dtype=np.float32))
    en = np.ascontiguousarray(np.asarray(inputs["end_transitions"], dtype=np.float32))
    tr = np.ascontiguousarray(np.asarray(inputs["transitions"], dtype=np.float32))
    tinit = np.tile(tr.ravel(), (128, 1)).astype(np.float32)
    tinit[0::C, :] = np.tile(st, 4)[None, :]
    rc = np.zeros((128, 64), dtype=np.float32)
    rc[:, 0:4] = [4.0, 3.0, 2.0, 1.0]
    rc[:, 4:8] = [0.0, 1.0, 2.0, 3.0]
    rc[:, 8:24] = np.eye(4, dtype=np.float32).ravel()[None, :]
    rc[:, 24:28] = en[None, :]
    rc[:, 28:44] = tr.ravel()[None, :]
    rc[:, 44:60] = tinit
    rc[:, 60:64] = bb[None, :]
    identc = np.eye(128, dtype=np.float32)
    return [{
        "sentences": sent[c * BC:(c + 1) * BC],
        "W": W, "identc": identc, "rowconsts": rc,
    } for c in range(NCORES)]


def kernel(**inputs):
    nc = _get_nc()
    in_maps = make_in_maps(inputs)
    res = run_bass_kernel_spmd(nc, in_maps, core_ids=list(range(NCORES)))
    tags = np.concatenate([res.results[c]["tags"] for c in range(NCORES)], axis=0)
    return tags.astype(np.int32)


if __name__ == "__main__":
    import reference
    inputs = {k: np.asarray(v) for k, v in reference.setup_inputs().items()}
    out = kernel(**inputs)
    print(out.shape, out.dtype, out[:2, :16])


# revision 15
# speedup vs baseline: 1.2527x; 1.0209x over previous
"""BERT-CRF Viterbi decode kernel for Trainium2 (Bass/Tile), 8-core data parallel.

Full inputs in, full outputs out. Internally shards batch B=64 across 8 cores
(8 sequences each). Per core, with scan rows r = b*16 + c (c = chunk of 32
timesteps):

  Stage A (u-tiled, fused with scan phase 1):
    for each scan step u (0..31), load sentences for all 128 rows at local
    step u, transpose h-chunks on PE, batched matmul (lhsT = W^T chunk [128,4],
    rhs = 4 steps' transposed sentences [128,512]) -> emissions^T in PSUM,
    fix-transpose back to [rows, 4], write directly into the SBUF scan tile.
    Phase 1 (chunk transfer-matrix recurrence) consumes each step's emissions
    as they land, hidden under stage A's PE/DMA time.
  Phase 2: boundary scores across chunks (sequential over 16, rows 0..7).
  Phase 3: all scores from boundaries + stored prefix matrices (2 big ops).
  Phase 4: backpointer one-hots, first-argmax semantics (6 big ops).
  Phase 5: one-hot matrix backtracking (no gathers).
"""
import sys
for p in ("/opt/trn_rl_repo", "/root/.axon_site/_ro/trn_rl_repo"):
    if p not in sys.path:
        sys.path.append(p)

import numpy as np
import concourse.bass as bass
import concourse.tile as tile
from concourse import mybir
from concourse.bass_utils import run_bass_kernel_spmd

F32 = mybir.dt.float32
F32R = mybir.dt.float32r
I32 = mybir.dt.int32
AX = mybir.AxisListType
OP = mybir.AluOpType

B, T, H, K = 64, 512, 768, 4
NCORES = 8
BC = B // NCORES          # 8 sequences per core
C, L = 16, 32             # chunks per sequence, steps per chunk
ROWS = BC * C             # 128 partition rows
HCH = H // 128            # 6 h-chunks
UG = 4                    # steps per u-group (batched matmul width 4*128=512)

_NC_CACHE = {}


def build_nc():
    nc = bass.Bass()
    sent = nc.declare_dram_parameter("sentences", [BC, T, H], F32, isOutput=False)
    Wd = nc.declare_dram_parameter("W", [K, H], F32, isOutput=False)
    identd = nc.declare_dram_parameter("identc", [128, 128], F32, isOutput=False)
    # rowconsts[128, 64]: wfirst | iw | ident4 | end | ttr | tinit | bias
    rcd = nc.declare_dram_parameter("rowconsts", [128, 64], F32, isOutput=False)
    tagsd = nc.declare_dram_parameter("tags", [BC, T], I32, isOutput=True)

    with tile.TileContext(nc) as tc:
        with tc.tile_pool(name="singles", bufs=1) as singles, \
             tc.tile_pool(name="sent_pool", bufs=6) as sent_pool, \
             tc.tile_pool(name="st_pool", bufs=2) as st_pool, \
             tc.tile_pool(name="tmp_pool", bufs=2) as tmp_pool, \
             tc.tile_pool(name="ps_tr", bufs=3, space="PSUM") as ps_tr, \
             tc.tile_pool(name="ps_eT", bufs=2, space="PSUM") as ps_eT, \
             tc.tile_pool(name="ps_fix", bufs=2, space="PSUM") as ps_fix:

            # ---------- prefetch first sentence group ----------
            pre_sents = []
            for uu in range(UG):
                s_sb = sent_pool.tile([128, H], F32, tag="sent")
                src0 = bass.AP(
                    tensor=sent[:].tensor, offset=uu * H,
                    ap=[[T * H, BC], [L * H, C], [1, H]])
                nc.sync.dma_start(s_sb, src0)
                pre_sents.append(s_sb)

            # ---------- constants ----------
            ident = singles.tile([128, 128], F32)
            nc.sync.dma_start(ident, identd[:])
            rc = singles.tile([128, 64], F32)
            nc.sync.dma_start(rc, rcd[:])
            wfirst = rc[:, 0:4]
            iw4 = rc[:, 4:8]
            id4 = rc[:, 8:24]
            end_sb = rc[:, 24:28]
            ttr = rc[:, 28:44]
            tinit = rc[:, 44:60]
            bias4 = rc[:, 60:64]

            # ---------- W^T in SBUF: wt[p = h within chunk, ch, k] ----------
            w_raw = singles.tile([K, H], F32)
            nc.sync.dma_start(w_raw, Wd[:])
            wt_sb = singles.tile([128, HCH, K], F32)
            for ch in range(HCH):
                wt_ps = ps_fix.tile([128, K], F32, tag="fix")
                nc.tensor.transpose(wt_ps, w_raw[:, ch * 128:(ch + 1) * 128],
                                    ident[0:K, 0:K])
                nc.scalar.copy(wt_sb[:, ch, :], wt_ps)

            # scan emissions tile, written directly by stage A
            emsc = singles.tile([128, L * K], F32)
            emv = emsc.rearrange("p (u j) -> p u j", u=L)

            # phase-1 state: prefix transfer matrices Apre[row, u, i, j]
            Apre = singles.tile([128, L, 4, 4], F32)

            # views
            ttrT_v = ttr.rearrange("p (k j) -> p k j", k=4).transpose([0, 2, 1])  # [p,j,k] = trans[k,j]
            ttr_ji = ttr.rearrange("p (i j) -> p i j", i=4).transpose([0, 2, 1])  # [p,j,i] = trans[i,j]

            # ---------- Stage A (u-tiled) fused with phase 1 ----------
            sA = nc.named_scope("stageA")
            sA.__enter__()
            for g in range(L // UG):
                if g == 0:
                    sents = pre_sents
                else:
                    sents = []
                    for uu in range(UG):
                        u = g * UG + uu
                        s_sb = sent_pool.tile([128, H], F32, tag="sent")
                        # row (b*16+c) <- sentences[b, c*32 + u, :]
                        src = bass.AP(
                            tensor=sent[:].tensor, offset=u * H,
                            ap=[[T * H, BC], [L * H, C], [1, H]])
                        nc.sync.dma_start(s_sb, src)
                        sents.append(s_sb)
                # transposes: sT[p=h, ch, uu, rows]
                sT_sb = st_pool.tile([128, HCH, UG, 128], F32)
                for ch in range(HCH):
                    trp = ps_tr.tile([128, UG * 128], F32, tag="trps")
                    for uu in range(UG):
                        nc.tensor.transpose(
                            trp[:, uu * 128:(uu + 1) * 128],
                            sents[uu][:, ch * 128:(ch + 1) * 128],
                            ident)
                    nc.scalar.copy(
                        sT_sb[:, ch, :, :].rearrange("p a b -> p (a b)"), trp)
                # batched matmuls: out eT[k, uu*128+row] accum over ch
                eT_ps = ps_eT.tile([4, UG * 128], F32, tag="eT")
                for ch in range(HCH):
                    nc.tensor.matmul(
                        eT_ps, wt_sb[:, ch, :],
                        sT_sb[:, ch, :, :].rearrange("p a b -> p (a b)"),
                        start=(ch == 0), stop=(ch == HCH - 1))
                eT_sb = st_pool.tile([4, UG * 128], F32, tag="eTsb")
                nc.scalar.copy(eT_sb, eT_ps)
                # fix-transpose each uu back to [rows, 4] and land in emsc
                for uu in range(UG):
                    u = g * UG + uu
                    fx = ps_fix.tile([128, K], F32, tag="fix")
                    nc.tensor.transpose(
                        fx, eT_sb[:, uu * 128:(uu + 1) * 128], ident[0:K, 0:K])
                    nc.scalar.copy(emsc[:, u * 4:(u + 1) * 4], fx)
                # bias (reference adds b last): emsc[:, g] += b
                nc.vector.tensor_tensor(
                    emv[:, g * UG:(g + 1) * UG, :],
                    emv[:, g * UG:(g + 1) * UG, :],
                    bias4.unsqueeze(1).to_broadcast((128, UG, 4)),
                    OP.add)
                # ---- phase 1 steps for this group ----
                for uu in range(UG):
                    u = g * UG + uu
                    if u == 0:
                        nc.vector.tensor_tensor(
                            Apre[:, 0, :, :],
                            tinit.rearrange("p (i j) -> p i j", i=4),
                            emv[:, 0, :].unsqueeze(1).to_broadcast((128, 4, 4)),
                            OP.add)
                    else:
                        p1tmp = tmp_pool.tile([128, 4, 4, 4], F32, tag="p1tmp")
                        # tmp[i,j,k] = A[i,k] + trans[k,j]
                        nc.vector.tensor_tensor(
                            p1tmp,
                            Apre[:, u - 1, :, :].unsqueeze(2).to_broadcast((128, 4, 4, 4)),
                            ttrT_v.unsqueeze(1).to_broadcast((128, 4, 4, 4)),
                            OP.add)
                        p1red = tmp_pool.tile([128, 4, 4], F32, tag="p1red")
                        nc.vector.reduce_max(p1red, p1tmp, axis=AX.X)
                        nc.vector.tensor_tensor(
                            Apre[:, u, :, :], p1red,
                            emv[:, u, :].unsqueeze(1).to_broadcast((128, 4, 4)), OP.add)
            sA.__exit__(None, None, None)

            # regroup A_c = Apre[:, L-1] to by-b layout [8, C*16]
            _sp2 = nc.named_scope("p2")
            _sp2.__enter__()
            abyb = singles.tile([BC, C * 16], F32)
            nc.sync.dma_start(abyb, Apre[:, L - 1, :, :].rearrange("p a b -> p (a b)"))
            abv = abyb.rearrange("p (c i j) -> p c i j", c=C, i=4)

            # ----- phase 2: boundary scores sbound[8, (C+1)*4], slot0 = 0 -----
            sbound = singles.tile([BC, (C + 1) * 4], F32)
            nc.vector.memset(sbound[:, 0:4], 0.0)
            sbv = sbound.rearrange("p (c j) -> p c j", c=C + 1)
            for c in range(C):
                p2tmp = tmp_pool.tile([BC, 4, 4], F32, tag="p2tmp")
                # tmp[j,i] = s[i] + A_c[i,j]
                nc.vector.tensor_tensor(
                    p2tmp,
                    sbv[:, c, :].unsqueeze(1).to_broadcast((BC, 4, 4)),
                    abv[:, c, :, :].transpose([0, 2, 1]),
                    OP.add)
                nc.vector.reduce_max(sbv[:, c + 1, :], p2tmp, axis=AX.X)
            _sp2.__exit__(None, None, None)

            # ----- phase 3 (parallel): scores[128, (L+1)*4] from boundary + Apre -----
            _sp3 = nc.named_scope("p3")
            _sp3.__enter__()
            scores = singles.tile([128, (L + 1) * 4], F32)
            nc.sync.dma_start(scores[:, 0:4], sbound[:, 0:C * 4])
            scv = scores.rearrange("p (u i) -> p u i", u=L + 1)
            p3tmp = singles.tile([128, L, 4, 4], F32)   # [u, j, i]
            nc.vector.tensor_tensor(
                p3tmp,
                scores[:, 0:4].unsqueeze(1).unsqueeze(1).to_broadcast((128, L, 4, 4)),
                Apre.transpose([0, 1, 3, 2]),
                OP.add)
            nc.vector.reduce_max(scv[:, 1:, :], p3tmp, axis=AX.X)
            _sp3.__exit__(None, None, None)

            # ----- phase 4: backpointer one-hots Pall[128, L, j, i] -----
            _sp4 = nc.named_scope("p4")
            _sp4.__enter__()
            cand = singles.tile([128, L, 4, 4], F32)
            nc.vector.tensor_tensor(
                cand,
                scv[:, 0:L, :].unsqueeze(2).to_broadcast((128, L, 4, 4)),
                ttr_ji.unsqueeze(1).to_broadcast((128, L, 4, 4)),
                OP.add)
            mxP = tmp_pool.tile([128, L, 4], F32, tag="mxP")
            nc.vector.reduce_max(mxP, cand, axis=AX.X)
            eqP = singles.tile([128, L, 4, 4], F32)
            nc.vector.tensor_tensor(eqP, cand, mxP.unsqueeze(3).to_broadcast((128, L, 4, 4)), OP.is_equal)
            nc.vector.tensor_tensor(
                eqP, eqP,
                wfirst.unsqueeze(1).unsqueeze(1).to_broadcast((128, L, 4, 4)),
                OP.mult)
            nc.vector.reduce_max(mxP, eqP, axis=AX.X)
            Pall = singles.tile([128, L, 4, 4], F32)
            nc.vector.tensor_tensor(Pall, eqP, mxP.unsqueeze(3).to_broadcast((128, L, 4, 4)), OP.is_equal)
            _sp4.__exit__(None, None, None)

            # ----- best_last one-hot on rows 0..7 -----
            ebyb = singles.tile([BC, C * 4], F32)
            ebv = ebyb.rearrange("p (c j) -> p c j", c=C)
            fin = tmp_pool.tile([BC, 4], F32, tag="fin")
            nc.vector.tensor_add(fin, sbv[:, C, :], end_sb[0:BC, :])
            mxf = tmp_pool.tile([BC, 1], F32, tag="mxf")
            nc.vector.reduce_max(mxf, fin, axis=AX.X)
            eqf = tmp_pool.tile([BC, 4], F32, tag="eqf")
            nc.vector.tensor_tensor(eqf, fin, mxf.to_broadcast((BC, 4)), OP.is_equal)
            nc.vector.tensor_tensor(eqf, eqf, wfirst[0:BC, :], OP.mult)
            nc.vector.reduce_max(mxf, eqf, axis=AX.X)
            nc.vector.tensor_tensor(ebv[:, C - 1, :], eqf, mxf.to_broadcast((BC, 4)), OP.is_equal)

            # ----- phase 5b: suffix maps Sall[128, L, x, i] + Ofull -----
            _sp5b = nc.named_scope("p5b")
            _sp5b.__enter__()
            Sall = singles.tile([128, L, 4, 4], F32)
            nc.vector.tensor_copy(Sall[:, L - 1, :, :], id4.rearrange("p (x i) -> p x i", x=4))
            for u in range(L - 2, -2, -1):
                p5tmp = tmp_pool.tile([128, 4, 4, 4], F32, tag="p5tmp")
                # tmp[x,i,y] = S_{u+1}[x,y] * P_{u+1}[y,i]
                nc.vector.tensor_tensor(
                    p5tmp,
                    Sall[:, u + 1, :, :].unsqueeze(2).to_broadcast((128, 4, 4, 4)),
                    Pall[:, u + 1, :, :].transpose([0, 2, 1]).unsqueeze(1).to_broadcast((128, 4, 4, 4)),
                    OP.mult)
                if u >= 0:
                    nc.vector.reduce_sum(Sall[:, u, :, :], p5tmp, axis=AX.X)
                else:
                    Ofull = singles.tile([128, 16], F32)
                    nc.vector.reduce_sum(Ofull.rearrange("p (x i) -> p x i", x=4),
                                         p5tmp, axis=AX.X)
            _sp5b.__exit__(None, None, None)

            # regroup Ofull to by-b [8, C*16]
            _sp5c = nc.named_scope("p5c")
            _sp5c.__enter__()
            obyb = singles.tile([BC, C * 16], F32)
            nc.sync.dma_start(obyb, Ofull)

            obv = obyb.rearrange("p (c x i) -> p c x i", c=C, x=4)

            # ----- phase 5c: boundary tags backward -----
            for c in range(C - 1, 0, -1):
                p5ctmp = tmp_pool.tile([BC, 4, 4], F32, tag="p5ctmp")
                # tmp[i,x] = E_c[x] * Ofull_c[x,i]
                nc.vector.tensor_tensor(
                    p5ctmp,
                    ebv[:, c, :].unsqueeze(1).to_broadcast((BC, 4, 4)),
                    obv[:, c, :, :].transpose([0, 2, 1]),
                    OP.mult)
                nc.vector.reduce_sum(ebv[:, c - 1, :], p5ctmp, axis=AX.X)

            # broadcast E to rows: ebc[128, 4], row b*16+c = E_c[b]
            ebc = singles.tile([128, 4], F32)
            nc.sync.dma_start(ebc, ebyb)
            _sp5c.__exit__(None, None, None)

            # ----- phase 5d: tags -----
            _sp5d = nc.named_scope("p5d")
            _sp5d.__enter__()
            G = tmp_pool.tile([128, 4, 4], F32, tag="G")
            nc.vector.tensor_tensor(
                G,
                ebc.unsqueeze(2).to_broadcast((128, 4, 4)),
                iw4.unsqueeze(1).to_broadcast((128, 4, 4)),
                OP.mult)
            p5dtmp = singles.tile([128, L, 4, 4], F32)
            nc.vector.tensor_tensor(
                p5dtmp, Sall,
                G.unsqueeze(1).to_broadcast((128, L, 4, 4)),
                OP.mult)
            tagf = tmp_pool.tile([128, L], F32, tag="tagf")
            nc.vector.reduce_sum(tagf, p5dtmp.rearrange("p u x i -> p u (x i)"), axis=AX.X)
            tagi = tmp_pool.tile([128, L], I32, tag="tagi")
            nc.vector.tensor_copy(tagi, tagf)
            nc.sync.dma_start(tagsd[:].rearrange("b (c t) -> b c t", c=C), tagi)
            _sp5d.__exit__(None, None, None)

    return nc


def _split_multi_waits(nc):
    """Walrus (bass2jax path) allows very few embedded sync waits per
    instruction (PE matmul: exactly 1). Hoist multi-waits onto standalone
    single-wait InstDrain instructions on the same engine, preserving order."""
    for f in nc.m.functions:
        for blk in f.blocks:
            insts = blk.instructions
            i = 0
            while i < len(insts):
                ins = insts[i]
                si = ins.sync_info
                w = list(si.on_wait) if (si is not None and si.on_wait) else []
                if len(w) >= 2:
                    for k, wait in enumerate(w):
                        d = mybir.InstEventSemaphore(
                            name=nc.get_next_instruction_name(), ins=[], outs=[])
                        d.engine = ins.engine
                        d.sync_info = mybir.SyncInfo(on_wait=[wait], on_update=[])
                        insts.insert(i + k, d)
                    i += len(w)
                    ins.sync_info = mybir.SyncInfo(
                        on_wait=[], on_update=list(si.on_update or []))
                i += 1


def _get_nc():
    if "nc" not in _NC_CACHE:
        nc = build_nc()
        _split_multi_waits(nc)   # HW path only; CoreSim rejects raw drains
        _NC_CACHE["nc"] = nc
    return _NC_CACHE["nc"]


def make_in_maps(inputs):
    sent = np.ascontiguousarray(np.asarray(inputs["sentences"], dtype=np.float32))
    W = np.ascontiguousarray(np.asarray(inputs["W"], dtype=np.float32))
    bb = np.ascontiguousarray(np.asarray(inputs["b"], dtype=np.float32))
    st = np.ascontiguousarray(np.asarray(inputs["start_transitions"], dtype=np.float32))
    en = np.ascontiguousarray(np.asarray(inputs["end_transitions"], dtype=np.float32))
    tr = np.ascontiguousarray(np.asarray(inputs["transitions"], dtype=np.float32))
    tinit = np.tile(tr.ravel(), (128, 1)).astype(np.float32)
    tinit[0::C, :] = np.tile(st, 4)[None, :]
    rc = np.zeros((128, 64), dtype=np.float32)
    rc[:, 0:4] = [4.0, 3.0, 2.0, 1.0]
    rc[:, 4:8] = [0.0, 1.0, 2.0, 3.0]
    rc[:, 8:24] = np.eye(4, dtype=np.float32).ravel()[None, :]
    rc[:, 24:28] = en[None, :]
    rc[:, 28:44] = tr.ravel()[None, :]
    rc[:, 44:60] = tinit
    rc[:, 60:64] = bb[None, :]
    identc = np.eye(128, dtype=np.float32)
    return [{
        "sentences": sent[c * BC:(c + 1) * BC],
        "W": W, "identc": identc, "rowconsts": rc,
    } for c in range(NCORES)]


def kernel(**inputs):
    nc = _get_nc()
    in_maps = make_in_maps(inputs)
    res = run_bass_kernel_spmd(nc, in_maps, core_ids=list(range(NCORES)))
    tags = np.concatenate([res.results[c]["tags"] for c in range(NCORES)], axis=0)
    return tags.astype(np.int32)


if __name__ == "__main__":
    import reference
    inputs = {k: np.asarray(v) for k, v in reference.setup_inputs().items()}
    out = kernel(**inputs)
    print(out.shape, out.dtype, out[:2, :16])
